# revision 26
# baseline (speedup 1.0000x reference)
"""Trainium2 Bass kernel for nn_Attention_43946105373274.

Causal multi-head attention with rotary embeddings applied to q, k and v.
B=2, N=2048, DIM=1024, H=16, DH=64, f32.

Sharding: 8 cores = (2 batches) x (4 head-groups of 4 heads).
Each core computes the qkv projection for its heads (w_qkv column-shard),
full causal attention for its heads, and a partial output projection
(w_out row-shard).  The host sums the 4 partials per batch and adds the
bias (the "all-reduce" of the output projection) — full inputs in, full
output out.

Structure (per core): phases are interleaved at the emission level so
every engine queue stays busy:
  seg 1: x^T prepared host-side (layout repack + bf16 cast) -> fast
         contiguous DMAs; cos/sin built on device (PE transpose-first,
         range-reduce on gpsimd, Sin on Act, bf16 tables).
  seg 2: qkv projection chains for head-pair 0 (software-pipelined
         rotate-half matmul + bf16 vector combine).
  seg 3: attention for heads 0-1 (S^T matmuls -> Exp on Act -> causal
         mask via gpsimd affine_select -> AV with a ones-column for the
         row sums) interleaved with the pair-1 qkv chains as tensor
         filler.
  seg 4: attention for heads 2-3 interleaved with the output projection
         (bf16 partials DMA'd out; host sums partials + bias).
"""

import sys
import numpy as np

if "/opt/trn_rl_repo" not in sys.path:
    sys.path.insert(0, "/opt/trn_rl_repo")

B, N, DIM, H, DH = 2, 2048, 1024, 16, 64
HPC = 4                     # heads per core
NCORES = 8
SCALE = DH ** -0.5
NT = N // 128               # 16 row tiles
KB = DIM // 128             # 8 contraction blocks
CW = 512                    # i-chunk width
NCH = N // CW               # 4 chunks
JW = 3 * HPC * DH           # 768 qkv columns per core

_CACHE = {}


def _build_program():
    import concourse.bass as bass  # noqa: F401
    import concourse.mybir as mybir
    import concourse.tile as tile
    from concourse import bacc

    F32 = mybir.dt.float32
    F32R = mybir.dt.float32r
    BF16 = mybir.dt.bfloat16
    AF = mybir.ActivationFunctionType
    OP = mybir.AluOpType

    nc = bacc.Bacc("TRN2", target_bir_lowering=False, debug=False,
                   num_devices=NCORES)

    xbT = nc.dram_tensor("xbT", [128, NCH * KB * CW], BF16, kind="ExternalInput")
    wqkv = nc.dram_tensor("wqkv", [6 * 128, DIM], BF16, kind="ExternalInput")
    wout = nc.dram_tensor("wout", [128, 2 * DIM], BF16, kind="ExternalInput")
    freqs = nc.dram_tensor("freqs", [N, DH], F32R, kind="ExternalInput")
    identR = nc.dram_tensor("identR", [128, 128], F32R, kind="ExternalInput")
    sgnD = nc.dram_tensor("sgnD", [64, 1], F32, kind="ExternalInput")
    maskD = nc.dram_tensor("maskD", [128, 128], BF16, kind="ExternalInput")
    identB = nc.dram_tensor("identB", [128, 128], BF16, kind="ExternalInput")
    outD = nc.dram_tensor("out", [N, DIM], BF16, kind="ExternalOutput")

    MAGIC = 12582912.0          # 1.5 * 2**23: float32 round-to-nearest trick
    TWO_PI = float(2 * np.pi)

    with tile.TileContext(nc) as tc:
        with tc.tile_pool(name="pc", bufs=1) as pc, \
             tc.tile_pool(name="pw", bufs=1) as pw, \
             tc.tile_pool(name="pxT", bufs=1) as pxT, \
             tc.tile_pool(name="pqk", bufs=4) as pqk, \
             tc.tile_pool(name="pv", bufs=4) as pv, \
             tc.tile_pool(name="psb", bufs=2) as psb, \
             tc.tile_pool(name="ppt", bufs=6) as ppt, \
             tc.tile_pool(name="poT", bufs=2) as poT, \
             tc.tile_pool(name="pnm", bufs=2) as pnm, \
             tc.tile_pool(name="pout", bufs=1) as pout, \
             tc.tile_pool(name="psA", bufs=2, space="PSUM") as psA, \
             tc.tile_pool(name="ps5", bufs=2, space="PSUM") as ps5, \
             tc.tile_pool(name="psT", bufs=1, space="PSUM") as psT:

            # ---------------- DMAs (priority order) --------------------------
            identb = pc.tile([128, 128], BF16, tag="identb")
            nc.sync.dma_start(identb[:], identB[:])
            ident = pc.tile([128, 128], F32R, tag="ident")
            nc.sync.dma_start(ident[:], identR[:])
            sgn = pc.tile([64, 1], F32, tag="sgn")
            nc.sync.dma_start(sgn[:], sgnD[:])
            msk = pc.tile([128, 128], BF16, tag="msk")
            nc.sync.dma_start(msk[:], maskD[:])
            # w packed host-side as [jt, p, kb, j] so each jt slice lands in
            # one 2KB-per-partition contiguous DMA
            w_all = pw.tile([128, 6, KB, 128], BF16, tag="w")
            wv = wqkv[:].rearrange("(t p) (k j) -> p t k j", p=128, k=KB)

            def dma_w(jt):
                nc.sync.dma_start(w_all[:, jt, :, :], wv[:, jt, :, :])

            # x^T chunk-major: [p][ch][kb][i] so each chunk is one
            # 128-descriptor contiguous DMA
            xT = pxT.tile([128, NCH, KB, CW], BF16, tag="xT")
            xbTv = xbT[:].rearrange("p (c k i) -> p c k i", c=NCH, k=KB)
            ftile = pc.tile([128, NT * DH], F32R, tag="ftile")
            fv = freqs[:].rearrange("(p t) d -> p (t d)", p=128)
            dma_w(4)
            nc.sync.dma_start(ftile[:, 0:512], fv[:, 0:512])
            dma_w(0)
            nc.sync.dma_start(xT[:, 0], xbTv[:, 0])
            nc.sync.dma_start(ftile[:, 512:1024], fv[:, 512:1024])
            dma_w(2)
            nc.sync.dma_start(xT[:, 1], xbTv[:, 1])
            wo_all = pw.tile([128, 2, DIM], BF16, tag="wo")

            def dma_late():
                for ch in (2, 3):
                    nc.sync.dma_start(xT[:, ch], xbTv[:, ch])
                for jt in (5, 1, 3):
                    dma_w(jt)
                nc.sync.dma_start(wo_all[:],
                                  wout[:].rearrange("p (c j) -> p c j", c=2))

            # ---------------- PE warmup + PSUM init --------------------------
            wmu = psT.tile([128, CW], BF16, tag="pstr", bufs=1, name="wmu")
            for r in range(12):
                nc.tensor.transpose(wmu[:, (r % 4) * 128:(r % 4 + 1) * 128],
                                    identb[:], identb[:])
            for i in range(2):
                z = psA.tile([128, 1024], F32, tag="sps", name=f"zinit{i}")
                nc.vector.memset(z[:], 0.0)

            # ---------------- small constants --------------------------------
            ones_f = pc.tile([128, 128], F32, tag="ones_f")
            nc.vector.memset(ones_f[:], 1.0)

            # persistent tensors
            qT = [pqk.tile([128, N], BF16, tag="qk", name=f"qT{i}") for i in range(2)]
            kT = [pqk.tile([128, N], BF16, tag="qk", name=f"kT{i}") for i in range(2)]
            # V tiles: [128, 65] per (head, row-tile); col 64 = ones
            vt = [pv.tile([128, NT * (DH + 1)], BF16, tag="v", name=f"vt{h}", bufs=4)
                  for h in range(HPC)]
            for h in range(HPC):
                vv = vt[h][:].rearrange("p (t c) -> p t c", c=DH + 1)
                nc.vector.tensor_copy(vv[:, :, DH:DH + 1],
                                      ones_f[:, 0:NT].unsqueeze(2))
            oT = [poT.tile([128, N], BF16, tag="oT", name=f"oT{i}") for i in range(2)]

            # ---------------- trig prep: cos/sin in [d, n] bf16 --------------
            # transpose freqs tiles first -> arg [128=(sin:0-63, cos:64-127), n]
            argT = pout.tile([128, N], F32, tag="argT", bufs=1)
            kt = pout.tile([128, N], F32, tag="kt", bufs=1)
            trigb = pc.tile([128, N], BF16, tag="trigb")
            sinb = pc.tile([128, N], BF16, tag="sinb")
            cosb = pc.tile([128, N], BF16, tag="cosb")
            for i in range(2):
                cl = slice(i * 1024, (i + 1) * 1024)
                fps = psA.tile([128, 1024], F32R, tag="sps", name=f"fps{i}")
                for t in range(8):
                    tt = i * 8 + t
                    nc.tensor.transpose(fps[0:64, t * 128:(t + 1) * 128],
                                        ftile[:, tt * DH:(tt + 1) * DH],
                                        ident[:])
                dsv = argT[:].rearrange("q (p t) -> q p t", t=NT)[:, :, i * 8:(i + 1) * 8]
                ssv = fps[0:64, :].rearrange("d (t p) -> d p t", p=128)
                nc.vector.tensor_scalar_mul(dsv[0:64], ssv, sgn[:, 0:1])
                nc.vector.tensor_scalar_add(dsv[64:128], ssv, float(np.pi / 2))
                nc.scalar.activation(kt[:, cl], argT[:, cl], AF.Copy,
                                     scale=float(1.0 / TWO_PI), bias=MAGIC)
                nc.scalar.activation(kt[:, cl], kt[:, cl], AF.Copy, bias=-MAGIC)
                nc.vector.scalar_tensor_tensor(argT[:, cl], kt[:, cl], -TWO_PI,
                                               argT[:, cl],
                                               op0=OP.mult, op1=OP.add)
                nc.scalar.activation(trigb[:, cl], argT[:, cl], AF.Sin)
                nc.vector.tensor_copy(sinb[0:64, cl], trigb[0:64, cl])
                nc.vector.tensor_copy(sinb[64:128, cl], trigb[0:64, cl])
                nc.vector.tensor_copy(cosb[0:64, cl], trigb[64:128, cl])
                nc.vector.tensor_copy(cosb[64:128, cl], trigb[64:128, cl])

            # ---------------- qkv chain machinery ----------------------------
            # A chain = qkv projection for one (jt, 512-col chunk), with the
            # rotate-half matmul + combine software-pipelined one chain behind
            # so the tensor queue never blocks on the cast round-trip.
            state = {"pending": None}

            def finish_rotary(jt, ch, t_sb, vec_cast):
                # rotate_half = partition pair swap (sign folded into sinb)
                r_sb = psb.tile([128, CW], BF16, tag="rsb", bufs=4)
                rv = r_sb[:].rearrange("(a two) w -> a two w", two=2)
                tv = t_sb[:].rearrange("(a two) w -> a two w", two=2)
                nc.sync.dma_start(rv[:, 0, :], tv[:, 1, :])
                nc.sync.dma_start(rv[:, 1, :], tv[:, 0, :])
                csl = cosb[:, ch * CW:(ch + 1) * CW]
                ssl = sinb[:, ch * CW:(ch + 1) * CW]
                tmp = psb.tile([128, CW], BF16, tag="tmp", bufs=4)
                nc.vector.tensor_mul(tmp[:], t_sb[:], csl)
                rs = psb.tile([128, CW], BF16, tag="rs2", bufs=4)
                nc.vector.tensor_mul(rs[:], r_sb[:], ssl)
                if jt < 4:  # q or k -> straight into qT/kT
                    dst = qT[jt] if jt < 2 else kT[jt - 2]
                    nc.vector.tensor_add(dst[:, ch * CW:(ch + 1) * CW],
                                         tmp[:], rs[:])
                else:       # v -> rotate then transpose into V tiles
                    v_sb = psb.tile([128, CW], BF16, tag="vsb", bufs=2)
                    nc.vector.tensor_add(v_sb[:], tmp[:], rs[:])
                    pair = jt - 4
                    vps = psT.tile([128, CW], BF16, tag="pstr", bufs=1,
                                   name=f"vps{jt}_{ch}")
                    for rt in range(4):
                        nc.tensor.transpose(
                            vps[:, rt * 128:(rt + 1) * 128],
                            v_sb[:, rt * 128:(rt + 1) * 128],
                            identb[:])
                    vpsv = vps[:].rearrange("p (t hh d) -> p t hh d", t=4, hh=2)
                    for hh in range(2):
                        h = pair * 2 + hh
                        dstv = vt[h][:].rearrange("p (t c) -> p t c", c=DH + 1)[
                            :, ch * 4:(ch + 1) * 4, 0:DH]
                        nc.vector.tensor_copy(dstv, vpsv[:, :, hh, :])

            def emit_chain(jt, ch, vec_cast):
                qps = ps5.tile([128, CW], F32, tag="pchain", bufs=2,
                               name=f"qps{jt}_{ch}")
                for kb in range(KB):
                    nc.tensor.matmul(
                        qps[:], w_all[:, jt, kb, :],
                        xT[:, ch, kb, :],
                        start=(kb == 0), stop=(kb == KB - 1))
                if state["pending"] is not None:
                    finish_rotary(*state["pending"])
                t_sb = psb.tile([128, CW], BF16, tag="tsb", bufs=4)
                if vec_cast:
                    nc.vector.tensor_copy(t_sb[:], qps[:])
                else:
                    nc.scalar.copy(t_sb[:], qps[:])
                state["pending"] = (jt, ch, t_sb, vec_cast)

            def flush_chain():
                if state["pending"] is not None:
                    finish_rotary(*state["pending"])
                    state["pending"] = None

            # ---------------- attention machinery ----------------------------
            def emit_norm(h, av_t, cc):
                pair, hh = h // 2, h % 2
                s_r = pnm.tile([1, CW], F32, tag="s_r", bufs=2,
                               name=f"s_r_{h}_{cc}")
                nc.vector.tensor_copy(s_r[:], av_t[DH:DH + 1, :])
                s_r2 = pnm.tile([1, CW], F32, tag="s_r2", bufs=2,
                                name=f"s_r2_{h}_{cc}")
                nc.vector.reciprocal_approx_fast(s_r2[:], s_r[:])
                rb = pnm.tile([64, CW], F32, tag="rb", bufs=2,
                              name=f"rb_{h}_{cc}")
                nc.gpsimd.partition_broadcast(rb[:], s_r2[:], channels=64)
                osl = oT[pair][hh * 64:(hh + 1) * 64, cc * CW:(cc + 1) * CW]
                nc.vector.tensor_mul(osl, av_t[0:DH, :], rb[:])

            def emit_group(h, c, grp, nj, av):
                pair, hh = h // 2, h % 2
                qh = qT[pair][hh * 64:(hh + 1) * 64, :]
                kh = kT[pair][hh * 64:(hh + 1) * 64, :]
                j0 = grp * 2
                sps = psA.tile([128, 1024], F32, tag="sps",
                               name=f"sps_{h}_{c}_{grp}")
                for g in range(2):
                    j = j0 + g
                    il0 = max(0, (j - 4 * c) * 128)   # causal: skip i < j cols
                    nc.tensor.matmul(
                        sps[:, g * 512 + il0:(g + 1) * 512],
                        kh[:, j * 128:(j + 1) * 128],
                        qh[:, c * CW + il0:(c + 1) * CW],
                        start=True, stop=True)
                pt = ppt.tile([128, 1024], BF16, tag="pt", bufs=6)
                nc.scalar.activation(pt[:], sps[:], AF.Exp, scale=SCALE)
                # The AV only reads i >= j columns, so only the 128x128
                # triangle block on the diagonal needs masking; the full
                # columns can feed AV straight from the exp.
                for g in range(2):
                    j = j0 + g
                    jrel = j - 4 * c
                    vblk = vt[h][:, j * (DH + 1):(j + 1) * (DH + 1)]
                    if jrel < 0:      # fully below diagonal
                        nc.tensor.matmul(av[:], vblk,
                                         pt[:, g * 512:(g + 1) * 512],
                                         start=(j == 0), stop=(j == nj - 1),
                                         skip_group_check=True)
                        continue
                    il0, il1 = jrel * 128, (jrel + 1) * 128
                    tri = pt[:, g * 512 + il0:g * 512 + il1]
                    nc.vector.tensor_mul(tri, tri, msk[:])
                    if j == 0:
                        # exactly one start=True per PSUM bank: single
                        # full-width matmul with the triangle pre-masked
                        nc.tensor.matmul(av[:], vblk,
                                         pt[:, g * 512:(g + 1) * 512],
                                         start=True, stop=False,
                                         skip_group_check=True)
                        continue
                    if il1 < CW:      # valid full columns, no mask needed
                        nc.tensor.matmul(av[:, il1:CW], vblk,
                                         pt[:, g * 512 + il1:(g + 1) * 512],
                                         start=False, stop=False,
                                         skip_group_check=True)
                    nc.tensor.matmul(av[:, il0:il1], vblk, tri,
                                     start=False, stop=(j == nj - 1),
                                     skip_group_check=True)

            # ---------------- unified schedule -------------------------------
            # Two chunk-chain sets up front, then attention starts; all
            # remaining qkv chains and the projection tiles feed in as
            # deferred tensor work, one item per attention group.
            for ch in (0, 1):
                for jt in (4, 0, 2):
                    emit_chain(jt, ch, vec_cast=False)
            dma_late()

            deferred = []
            for ch in (2, 3):
                for jt in (4, 0, 2):
                    deferred.append(("chain", jt, ch))
            for jt in (5, 3, 1):
                for ch in range(NCH):
                    deferred.append(("chain", jt, ch))
            deferred.append(("flush",))
            deferred = list(reversed(deferred))   # pop() from the end

            proj_queue = []

            def emit_proj_tile(nt_i):
                ot = pout.tile([128, DIM], BF16, tag="osb", bufs=3,
                               name=f"ot{nt_i}")
                for mh in range(2):
                    prjh = ps5.tile([128, CW], F32, tag="pchain", bufs=2,
                                    name=f"prj{nt_i}_{mh}")
                    for cb in range(2):
                        nc.tensor.matmul(
                            prjh[:],
                            oT[cb][:, nt_i * 128:(nt_i + 1) * 128],
                            wo_all[:, cb, mh * 512:(mh + 1) * 512],
                            start=(cb == 0), stop=(cb == 1))
                    if mh == 0:
                        nc.vector.tensor_copy(ot[:, 0:512], prjh[:])
                    else:
                        nc.scalar.copy(ot[:, 512:1024], prjh[:])
                nc.sync.dma_start(outD[nt_i * 128:(nt_i + 1) * 128, :], ot[:])

            def pop_deferred():
                if deferred:
                    item = deferred.pop()
                    if item[0] == "chain":
                        emit_chain(item[1], item[2], vec_cast=True)
                    else:
                        flush_chain()
                elif proj_queue:
                    emit_proj_tile(proj_queue.pop(0))

            gcnt = [0]

            def att_chunk(c, heads):
                nj = 4 * c + 4
                for h in heads:
                    av = ps5.tile([DH + 1, CW], F32, tag="pav", bufs=1,
                                  name=f"av_{h}_{c}")
                    for grp in range(nj // 2):
                        emit_group(h, c, grp, nj, av)
                        if gcnt[0] % 2 == 0:
                            pop_deferred()
                        gcnt[0] += 1
                    emit_norm(h, av, c)

            for c in range(NCH):
                att_chunk(c, (0, 1))
            for c in range(NCH):
                att_chunk(c, (2, 3))
                proj_queue.extend(range(4 * c, 4 * c + 4))
            while deferred or proj_queue:
                pop_deferred()

    nc.compile()
    return nc


def _get_program():
    if "nc" not in _CACHE:
        _CACHE["nc"] = _build_program()
    return _CACHE["nc"]


def make_in_maps(x, rotary_pos_emb, w_qkv, w_out, b_out):
    x = np.asarray(x, np.float32)
    rotary_pos_emb = np.ascontiguousarray(np.asarray(rotary_pos_emb, np.float32))
    w_qkv = np.asarray(w_qkv, np.float32)
    w_out = np.asarray(w_out, np.float32)

    import ml_dtypes
    bf16 = ml_dtypes.bfloat16
    ident = np.eye(128, dtype=np.float32)
    identb = np.eye(128).astype(bf16)
    sgn = np.where(np.arange(64) % 2 == 0, -1.0, 1.0).astype(np.float32)[:, None]

    # [p][ch][kb][i]: x^T chunk-major so device DMAs are contiguous/partition
    xT = [np.ascontiguousarray(
        x[b].T.reshape(KB, 128, NCH, CW).transpose(1, 2, 0, 3)
        .reshape(128, NCH * KB * CW)).astype(bf16) for b in range(B)]

    in_maps = []
    for c in range(NCORES):
        b = c // 4
        heads = [4 * (c % 4) + i for i in range(HPC)]
        # w_qkv column shard in j-tile order: q01,q23,k01,k23,v01,v23
        cols = []
        for t in range(3):            # q, k, v
            for h in heads:
                cols.append(w_qkv[:, t * H * DH + h * DH: t * H * DH + (h + 1) * DH])
        w_s = np.concatenate(cols, axis=1)             # [1024, 768]
        # repack to [jt, p, kb, j] rows so per-jt DMA slices are contiguous
        w_s = np.ascontiguousarray(
            w_s.reshape(KB, 128, 6, 128).transpose(2, 1, 0, 3).reshape(768, DIM))
        w_o = np.concatenate(
            [w_out[h * DH:(h + 1) * DH, :] for h in heads], axis=0)  # [256,1024]
        # repack to [p, cb, j]
        w_o = np.ascontiguousarray(
            w_o.reshape(2, 128, DIM).transpose(1, 0, 2).reshape(128, 2 * DIM))
        in_maps.append({
            "xbT": xT[b],
            "wqkv": w_s.astype(bf16),
            "wout": w_o.astype(bf16),
            "freqs": rotary_pos_emb,
            "identR": ident,
            "sgnD": sgn,
            "maskD": np.triu(np.ones((128, 128), np.float32)).astype(bf16),
            "identB": identb,
        })
    return in_maps


def _gather(res, b_out):
    out = np.zeros((B, N, DIM), np.float32)
    for c in range(NCORES):
        out[c // 4] += np.asarray(res[c]["out"]).astype(np.float32)
    out += np.asarray(b_out, np.float32)[None, None, :]
    return out


def kernel(x, rotary_pos_emb, w_qkv, w_out, b_out):
    from concourse.bass_utils import run_bass_kernel_spmd

    nc = _get_program()
    in_maps = make_in_maps(x, rotary_pos_emb, w_qkv, w_out, b_out)
    res = run_bass_kernel_spmd(nc, in_maps, list(range(NCORES))).results
    return _gather(res, b_out)


# revision 28
# speedup vs baseline: 1.0072x; 1.0072x over previous
"""Trainium2 Bass kernel for nn_Attention_43946105373274.

Causal multi-head attention with rotary embeddings applied to q, k and v.
B=2, N=2048, DIM=1024, H=16, DH=64, f32.

Sharding: 8 cores = (2 batches) x (4 head-groups of 4 heads).
Each core computes the qkv projection for its heads (w_qkv column-shard),
full causal attention for its heads, and a partial output projection
(w_out row-shard).  The host sums the 4 partials per batch and adds the
bias (the "all-reduce" of the output projection) — full inputs in, full
output out.

Structure (per core): phases are interleaved at the emission level so
every engine queue stays busy:
  seg 1: x^T prepared host-side (layout repack + bf16 cast) -> fast
         contiguous DMAs; cos/sin built on device (PE transpose-first,
         range-reduce on gpsimd, Sin on Act, bf16 tables).
  seg 2: qkv projection chains for head-pair 0 (software-pipelined
         rotate-half matmul + bf16 vector combine).
  seg 3: attention for heads 0-1 (S^T matmuls -> Exp on Act -> causal
         mask via gpsimd affine_select -> AV with a ones-column for the
         row sums) interleaved with the pair-1 qkv chains as tensor
         filler.
  seg 4: attention for heads 2-3 interleaved with the output projection
         (bf16 partials DMA'd out; host sums partials + bias).
"""

import sys
import numpy as np

if "/opt/trn_rl_repo" not in sys.path:
    sys.path.insert(0, "/opt/trn_rl_repo")

B, N, DIM, H, DH = 2, 2048, 1024, 16, 64
HPC = 4                     # heads per core
NCORES = 8
SCALE = DH ** -0.5
NT = N // 128               # 16 row tiles
KB = DIM // 128             # 8 contraction blocks
CW = 512                    # i-chunk width
NCH = N // CW               # 4 chunks
JW = 3 * HPC * DH           # 768 qkv columns per core

_CACHE = {}


def _build_program():
    import concourse.bass as bass  # noqa: F401
    import concourse.mybir as mybir
    import concourse.tile as tile
    from concourse import bacc

    F32 = mybir.dt.float32
    F32R = mybir.dt.float32r
    BF16 = mybir.dt.bfloat16
    AF = mybir.ActivationFunctionType
    OP = mybir.AluOpType

    nc = bacc.Bacc("TRN2", target_bir_lowering=False, debug=False,
                   num_devices=NCORES)

    xbT = nc.dram_tensor("xbT", [128, NCH * KB * CW], BF16, kind="ExternalInput")
    wqkv = nc.dram_tensor("wqkv", [6 * 128, DIM], BF16, kind="ExternalInput")
    wout = nc.dram_tensor("wout", [128, 2 * DIM], BF16, kind="ExternalInput")
    freqs = nc.dram_tensor("freqs", [N, DH], F32R, kind="ExternalInput")
    identR = nc.dram_tensor("identR", [128, 128], F32R, kind="ExternalInput")
    sgnD = nc.dram_tensor("sgnD", [64, 1], F32, kind="ExternalInput")
    maskD = nc.dram_tensor("maskD", [128, 128], BF16, kind="ExternalInput")
    identB = nc.dram_tensor("identB", [128, 128], BF16, kind="ExternalInput")
    outD = nc.dram_tensor("out", [N, DIM], BF16, kind="ExternalOutput")

    MAGIC = 12582912.0          # 1.5 * 2**23: float32 round-to-nearest trick
    TWO_PI = float(2 * np.pi)

    with tile.TileContext(nc) as tc:
        with tc.tile_pool(name="pc", bufs=1) as pc, \
             tc.tile_pool(name="pw", bufs=1) as pw, \
             tc.tile_pool(name="pxT", bufs=1) as pxT, \
             tc.tile_pool(name="pqk", bufs=4) as pqk, \
             tc.tile_pool(name="pv", bufs=4) as pv, \
             tc.tile_pool(name="psb", bufs=2) as psb, \
             tc.tile_pool(name="ppt", bufs=6) as ppt, \
             tc.tile_pool(name="poT", bufs=2) as poT, \
             tc.tile_pool(name="pnm", bufs=2) as pnm, \
             tc.tile_pool(name="pout", bufs=1) as pout, \
             tc.tile_pool(name="psA", bufs=2, space="PSUM") as psA, \
             tc.tile_pool(name="ps5", bufs=2, space="PSUM") as ps5, \
             tc.tile_pool(name="psT", bufs=1, space="PSUM") as psT:

            # ---------------- DMAs (priority order) --------------------------
            identb = pc.tile([128, 128], BF16, tag="identb")
            nc.sync.dma_start(identb[:], identB[:])
            ident = pc.tile([128, 128], F32R, tag="ident")
            nc.sync.dma_start(ident[:], identR[:])
            sgn = pc.tile([64, 1], F32, tag="sgn")
            nc.sync.dma_start(sgn[:], sgnD[:])
            msk = pc.tile([128, 128], BF16, tag="msk")
            nc.sync.dma_start(msk[:], maskD[:])
            # w packed host-side as [jt, p, kb, j] so each jt slice lands in
            # one 2KB-per-partition contiguous DMA
            w_all = pw.tile([128, 6, KB, 128], BF16, tag="w")
            wv = wqkv[:].rearrange("(t p) (k j) -> p t k j", p=128, k=KB)

            def dma_w(jt):
                nc.sync.dma_start(w_all[:, jt, :, :], wv[:, jt, :, :])

            # x^T chunk-major: [p][ch][kb][i] so each chunk is one
            # 128-descriptor contiguous DMA
            xT = pxT.tile([128, NCH, KB, CW], BF16, tag="xT")
            xbTv = xbT[:].rearrange("p (c k i) -> p c k i", c=NCH, k=KB)
            ftile = pc.tile([128, NT * DH], F32R, tag="ftile")
            fv = freqs[:].rearrange("(p t) d -> p (t d)", p=128)
            dma_w(4)
            nc.scalar.dma_start(ftile[:, 0:512], fv[:, 0:512])
            nc.scalar.dma_start(w_all[:, 0, :, :], wv[:, 0, :, :])
            nc.sync.dma_start(xT[:, 0], xbTv[:, 0])
            nc.scalar.dma_start(ftile[:, 512:1024], fv[:, 512:1024])
            nc.scalar.dma_start(w_all[:, 2, :, :], wv[:, 2, :, :])
            nc.sync.dma_start(xT[:, 1], xbTv[:, 1])
            wo_all = pw.tile([128, 2, DIM], BF16, tag="wo")

            def dma_late():
                for ch in (2, 3):
                    nc.sync.dma_start(xT[:, ch], xbTv[:, ch])
                for jt in (5, 1, 3):
                    dma_w(jt)
                nc.sync.dma_start(wo_all[:],
                                  wout[:].rearrange("p (c j) -> p c j", c=2))

            # ---------------- PE warmup + PSUM init --------------------------
            wmu = psT.tile([128, CW], BF16, tag="pstr", bufs=1, name="wmu")
            for r in range(12):
                nc.tensor.transpose(wmu[:, (r % 4) * 128:(r % 4 + 1) * 128],
                                    identb[:], identb[:])
            for i in range(2):
                z = psA.tile([128, 1024], F32, tag="sps", name=f"zinit{i}")
                nc.vector.memset(z[:], 0.0)

            # ---------------- small constants --------------------------------
            ones_f = pc.tile([128, 128], F32, tag="ones_f")
            nc.vector.memset(ones_f[:], 1.0)

            # persistent tensors
            qT = [pqk.tile([128, N], BF16, tag="qk", name=f"qT{i}") for i in range(2)]
            kT = [pqk.tile([128, N], BF16, tag="qk", name=f"kT{i}") for i in range(2)]
            # V tiles: [128, 65] per (head, row-tile); col 64 = ones
            vt = [pv.tile([128, NT * (DH + 1)], BF16, tag="v", name=f"vt{h}", bufs=4)
                  for h in range(HPC)]
            for h in range(HPC):
                vv = vt[h][:].rearrange("p (t c) -> p t c", c=DH + 1)
                nc.vector.tensor_copy(vv[:, :, DH:DH + 1],
                                      ones_f[:, 0:NT].unsqueeze(2))
            oT = [poT.tile([128, N], BF16, tag="oT", name=f"oT{i}") for i in range(2)]

            # ---------------- trig prep: cos/sin in [d, n] bf16 --------------
            # transpose freqs tiles first -> arg [128=(sin:0-63, cos:64-127), n]
            argT = pout.tile([128, N], F32, tag="argT", bufs=1)
            kt = pout.tile([128, N], F32, tag="kt", bufs=1)
            trigb = pc.tile([128, N], BF16, tag="trigb")
            sinb = pc.tile([128, N], BF16, tag="sinb")
            cosb = pc.tile([128, N], BF16, tag="cosb")
            for i in range(2):
                cl = slice(i * 1024, (i + 1) * 1024)
                fps = psA.tile([128, 1024], F32R, tag="sps", name=f"fps{i}")
                for t in range(8):
                    tt = i * 8 + t
                    nc.tensor.transpose(fps[0:64, t * 128:(t + 1) * 128],
                                        ftile[:, tt * DH:(tt + 1) * DH],
                                        ident[:])
                dsv = argT[:].rearrange("q (p t) -> q p t", t=NT)[:, :, i * 8:(i + 1) * 8]
                ssv = fps[0:64, :].rearrange("d (t p) -> d p t", p=128)
                nc.vector.tensor_scalar_mul(dsv[0:64], ssv, sgn[:, 0:1])
                nc.vector.tensor_scalar_add(dsv[64:128], ssv, float(np.pi / 2))
                nc.scalar.activation(kt[:, cl], argT[:, cl], AF.Copy,
                                     scale=float(1.0 / TWO_PI), bias=MAGIC)
                nc.scalar.activation(kt[:, cl], kt[:, cl], AF.Copy, bias=-MAGIC)
                nc.vector.scalar_tensor_tensor(argT[:, cl], kt[:, cl], -TWO_PI,
                                               argT[:, cl],
                                               op0=OP.mult, op1=OP.add)
                nc.scalar.activation(trigb[:, cl], argT[:, cl], AF.Sin)
                nc.vector.tensor_copy(sinb[0:64, cl], trigb[0:64, cl])
                nc.vector.tensor_copy(sinb[64:128, cl], trigb[0:64, cl])
                nc.vector.tensor_copy(cosb[0:64, cl], trigb[64:128, cl])
                nc.vector.tensor_copy(cosb[64:128, cl], trigb[64:128, cl])

            # ---------------- qkv chain machinery ----------------------------
            # A chain = qkv projection for one (jt, 512-col chunk), with the
            # rotate-half matmul + combine software-pipelined one chain behind
            # so the tensor queue never blocks on the cast round-trip.
            state = {"pending": None}

            def finish_rotary(jt, ch, t_sb, vec_cast):
                # rotate_half = partition pair swap (sign folded into sinb)
                r_sb = psb.tile([128, CW], BF16, tag="rsb", bufs=4)
                rv = r_sb[:].rearrange("(a two) w -> a two w", two=2)
                tv = t_sb[:].rearrange("(a two) w -> a two w", two=2)
                nc.sync.dma_start(rv[:, 0, :], tv[:, 1, :])
                nc.sync.dma_start(rv[:, 1, :], tv[:, 0, :])
                csl = cosb[:, ch * CW:(ch + 1) * CW]
                ssl = sinb[:, ch * CW:(ch + 1) * CW]
                tmp = psb.tile([128, CW], BF16, tag="tmp", bufs=4)
                nc.vector.tensor_mul(tmp[:], t_sb[:], csl)
                rs = psb.tile([128, CW], BF16, tag="rs2", bufs=4)
                nc.vector.tensor_mul(rs[:], r_sb[:], ssl)
                if jt < 4:  # q or k -> straight into qT/kT
                    dst = qT[jt] if jt < 2 else kT[jt - 2]
                    nc.vector.tensor_add(dst[:, ch * CW:(ch + 1) * CW],
                                         tmp[:], rs[:])
                else:       # v -> rotate then transpose into V tiles
                    v_sb = psb.tile([128, CW], BF16, tag="vsb", bufs=2)
                    nc.vector.tensor_add(v_sb[:], tmp[:], rs[:])
                    pair = jt - 4
                    vps = psT.tile([128, CW], BF16, tag="pstr", bufs=1,
                                   name=f"vps{jt}_{ch}")
                    for rt in range(4):
                        nc.tensor.transpose(
                            vps[:, rt * 128:(rt + 1) * 128],
                            v_sb[:, rt * 128:(rt + 1) * 128],
                            identb[:])
                    vpsv = vps[:].rearrange("p (t hh d) -> p t hh d", t=4, hh=2)
                    for hh in range(2):
                        h = pair * 2 + hh
                        dstv = vt[h][:].rearrange("p (t c) -> p t c", c=DH + 1)[
                            :, ch * 4:(ch + 1) * 4, 0:DH]
                        nc.vector.tensor_copy(dstv, vpsv[:, :, hh, :])

            def emit_chain(jt, ch, vec_cast):
                qps = ps5.tile([128, CW], F32, tag="pchain", bufs=2,
                               name=f"qps{jt}_{ch}")
                for kb in range(KB):
                    nc.tensor.matmul(
                        qps[:], w_all[:, jt, kb, :],
                        xT[:, ch, kb, :],
                        start=(kb == 0), stop=(kb == KB - 1))
                if state["pending"] is not None:
                    finish_rotary(*state["pending"])
                t_sb = psb.tile([128, CW], BF16, tag="tsb", bufs=4)
                if vec_cast:
                    nc.vector.tensor_copy(t_sb[:], qps[:])
                else:
                    nc.scalar.copy(t_sb[:], qps[:])
                state["pending"] = (jt, ch, t_sb, vec_cast)

            def flush_chain():
                if state["pending"] is not None:
                    finish_rotary(*state["pending"])
                    state["pending"] = None

            # ---------------- attention machinery ----------------------------
            def emit_norm(h, av_t, cc):
                pair, hh = h // 2, h % 2
                s_r = pnm.tile([1, CW], F32, tag="s_r", bufs=2,
                               name=f"s_r_{h}_{cc}")
                nc.vector.tensor_copy(s_r[:], av_t[DH:DH + 1, :])
                s_r2 = pnm.tile([1, CW], F32, tag="s_r2", bufs=2,
                                name=f"s_r2_{h}_{cc}")
                nc.vector.reciprocal_approx_fast(s_r2[:], s_r[:])
                rb = pnm.tile([64, CW], F32, tag="rb", bufs=2,
                              name=f"rb_{h}_{cc}")
                nc.gpsimd.partition_broadcast(rb[:], s_r2[:], channels=64)
                osl = oT[pair][hh * 64:(hh + 1) * 64, cc * CW:(cc + 1) * CW]
                nc.vector.tensor_mul(osl, av_t[0:DH, :], rb[:])

            def emit_group(h, c, grp, nj, av):
                pair, hh = h // 2, h % 2
                qh = qT[pair][hh * 64:(hh + 1) * 64, :]
                kh = kT[pair][hh * 64:(hh + 1) * 64, :]
                j0 = grp * 2
                sps = psA.tile([128, 1024], F32, tag="sps",
                               name=f"sps_{h}_{c}_{grp}")
                for g in range(2):
                    j = j0 + g
                    il0 = max(0, (j - 4 * c) * 128)   # causal: skip i < j cols
                    nc.tensor.matmul(
                        sps[:, g * 512 + il0:(g + 1) * 512],
                        kh[:, j * 128:(j + 1) * 128],
                        qh[:, c * CW + il0:(c + 1) * CW],
                        start=True, stop=True)
                pt = ppt.tile([128, 1024], BF16, tag="pt", bufs=6)
                nc.scalar.activation(pt[:], sps[:], AF.Exp, scale=SCALE)
                # The AV only reads i >= j columns, so only the 128x128
                # triangle block on the diagonal needs masking; the full
                # columns can feed AV straight from the exp.
                for g in range(2):
                    j = j0 + g
                    jrel = j - 4 * c
                    vblk = vt[h][:, j * (DH + 1):(j + 1) * (DH + 1)]
                    if jrel < 0:      # fully below diagonal
                        nc.tensor.matmul(av[:], vblk,
                                         pt[:, g * 512:(g + 1) * 512],
                                         start=(j == 0), stop=(j == nj - 1),
                                         skip_group_check=True)
                        continue
                    il0, il1 = jrel * 128, (jrel + 1) * 128
                    tri = pt[:, g * 512 + il0:g * 512 + il1]
                    nc.vector.tensor_mul(tri, tri, msk[:])
                    if j == 0:
                        # exactly one start=True per PSUM bank: single
                        # full-width matmul with the triangle pre-masked
                        nc.tensor.matmul(av[:], vblk,
                                         pt[:, g * 512:(g + 1) * 512],
                                         start=True, stop=False,
                                         skip_group_check=True)
                        continue
                    if il1 < CW:      # valid full columns, no mask needed
                        nc.tensor.matmul(av[:, il1:CW], vblk,
                                         pt[:, g * 512 + il1:(g + 1) * 512],
                                         start=False, stop=False,
                                         skip_group_check=True)
                    nc.tensor.matmul(av[:, il0:il1], vblk, tri,
                                     start=False, stop=(j == nj - 1),
                                     skip_group_check=True)

            # ---------------- unified schedule -------------------------------
            # Two chunk-chain sets up front, then attention starts; all
            # remaining qkv chains and the projection tiles feed in as
            # deferred tensor work, one item per attention group.
            for ch in (0, 1):
                for jt in (4, 0, 2):
                    emit_chain(jt, ch, vec_cast=False)
            dma_late()

            deferred = []
            for ch in (2, 3):
                for jt in (4, 0, 2):
                    deferred.append(("chain", jt, ch))
            for jt in (5, 3, 1):
                for ch in range(NCH):
                    deferred.append(("chain", jt, ch))
            deferred.append(("flush",))
            deferred = list(reversed(deferred))   # pop() from the end

            proj_queue = []

            def emit_proj_tile(nt_i):
                ot = pout.tile([128, DIM], BF16, tag="osb", bufs=3,
                               name=f"ot{nt_i}")
                for mh in range(2):
                    prjh = ps5.tile([128, CW], F32, tag="pchain", bufs=2,
                                    name=f"prj{nt_i}_{mh}")
                    for cb in range(2):
                        nc.tensor.matmul(
                            prjh[:],
                            oT[cb][:, nt_i * 128:(nt_i + 1) * 128],
                            wo_all[:, cb, mh * 512:(mh + 1) * 512],
                            start=(cb == 0), stop=(cb == 1))
                    if mh == 0:
                        nc.vector.tensor_copy(ot[:, 0:512], prjh[:])
                    else:
                        nc.scalar.copy(ot[:, 512:1024], prjh[:])
                nc.sync.dma_start(outD[nt_i * 128:(nt_i + 1) * 128, :], ot[:])

            def pop_deferred():
                if deferred:
                    item = deferred.pop()
                    if item[0] == "chain":
                        emit_chain(item[1], item[2], vec_cast=True)
                    else:
                        flush_chain()
                elif proj_queue:
                    emit_proj_tile(proj_queue.pop(0))

            gcnt = [0]

            def att_chunk(c, heads):
                nj = 4 * c + 4
                for h in heads:
                    av = ps5.tile([DH + 1, CW], F32, tag="pav", bufs=1,
                                  name=f"av_{h}_{c}")
                    for grp in range(nj // 2):
                        emit_group(h, c, grp, nj, av)
                        if gcnt[0] % 2 == 0:
                            pop_deferred()
                        gcnt[0] += 1
                    emit_norm(h, av, c)

            for c in range(NCH):
                att_chunk(c, (0, 1))
            for c in range(NCH):
                att_chunk(c, (2, 3))
                proj_queue.extend(range(4 * c, 4 * c + 4))
            while deferred or proj_queue:
                pop_deferred()

    nc.compile()
    return nc


def _get_program():
    if "nc" not in _CACHE:
        _CACHE["nc"] = _build_program()
    return _CACHE["nc"]


def make_in_maps(x, rotary_pos_emb, w_qkv, w_out, b_out):
    x = np.asarray(x, np.float32)
    rotary_pos_emb = np.ascontiguousarray(np.asarray(rotary_pos_emb, np.float32))
    w_qkv = np.asarray(w_qkv, np.float32)
    w_out = np.asarray(w_out, np.float32)

    import ml_dtypes
    bf16 = ml_dtypes.bfloat16
    ident = np.eye(128, dtype=np.float32)
    identb = np.eye(128).astype(bf16)
    sgn = np.where(np.arange(64) % 2 == 0, -1.0, 1.0).astype(np.float32)[:, None]

    # [p][ch][kb][i]: x^T chunk-major so device DMAs are contiguous/partition
    xT = [np.ascontiguousarray(
        x[b].T.reshape(KB, 128, NCH, CW).transpose(1, 2, 0, 3)
        .reshape(128, NCH * KB * CW)).astype(bf16) for b in range(B)]

    in_maps = []
    for c in range(NCORES):
        b = c // 4
        heads = [4 * (c % 4) + i for i in range(HPC)]
        # w_qkv column shard in j-tile order: q01,q23,k01,k23,v01,v23
        cols = []
        for t in range(3):            # q, k, v
            for h in heads:
                cols.append(w_qkv[:, t * H * DH + h * DH: t * H * DH + (h + 1) * DH])
        w_s = np.concatenate(cols, axis=1)             # [1024, 768]
        # repack to [jt, p, kb, j] rows so per-jt DMA slices are contiguous
        w_s = np.ascontiguousarray(
            w_s.reshape(KB, 128, 6, 128).transpose(2, 1, 0, 3).reshape(768, DIM))
        w_o = np.concatenate(
            [w_out[h * DH:(h + 1) * DH, :] for h in heads], axis=0)  # [256,1024]
        # repack to [p, cb, j]
        w_o = np.ascontiguousarray(
            w_o.reshape(2, 128, DIM).transpose(1, 0, 2).reshape(128, 2 * DIM))
        in_maps.append({
            "xbT": xT[b],
            "wqkv": w_s.astype(bf16),
            "wout": w_o.astype(bf16),
            "freqs": rotary_pos_emb,
            "identR": ident,
            "sgnD": sgn,
            "maskD": np.triu(np.ones((128, 128), np.float32)).astype(bf16),
            "identB": identb,
        })
    return in_maps


def _gather(res, b_out):
    out = np.zeros((B, N, DIM), np.float32)
    for c in range(NCORES):
        out[c // 4] += np.asarray(res[c]["out"]).astype(np.float32)
    out += np.asarray(b_out, np.float32)[None, None, :]
    return out


def kernel(x, rotary_pos_emb, w_qkv, w_out, b_out):
    from concourse.bass_utils import run_bass_kernel_spmd

    nc = _get_program()
    in_maps = make_in_maps(x, rotary_pos_emb, w_qkv, w_out, b_out)
    res = run_bass_kernel_spmd(nc, in_maps, list(range(NCORES))).results
    return _gather(res, b_out)
